# revision 25
# baseline (speedup 1.0000x reference)
"""nn_GRUBlock Trainium2 kernel: y = GRU2(gelu(GRU1(x))).

Self-contained: builds a Bass/Tile program, shards batch B=16 across 8
NeuronCores (B=2 per core), runs via run_bass_kernel_spmd, gathers the
full output.

Per-core program v2 (software-pipelined layers, fully unrolled steps):
  - L1 and L2 recurrences run as two concurrent chains; L2 lags L1 by one
    chunk of S steps. Their serial gate chains interleave in each other's
    engine-latency gaps.
  - per step, one PSUM tile [128, 12*NB] accumulates: xp injection
    (identity matmul), b_hn injection (ones matmul, n-gates only), and
    the 48 W_hh matmuls. Sigmoid reads PSUM directly.
  - gate chain: sigmoid(rz) -> t1=r*ps_n -> t2=t1+xp_n -> tanh ->
    u=(1-z)*n -> h'=u+z*h  (zb, zh computed off the critical path)
  - input-projection GEMMs per chunk; bias-add via ACT (per-partition
    bias); GELU multiplies on GPSIMD; xp stored fp16.
"""

from contextlib import ExitStack

import numpy as np

B, T, DIN, H = 16, 4096, 512, 512
N_CORES = 8
NB = B // N_CORES      # batch per core
S = 128                # chunk (steps)

_CACHE = {}
TRACE = False          # set by test.py to capture NTFF profile
TRACE_CORES = None
LAST_RESULTS = None    # BassKernelResults of last kernel() call


def _build(T_, S_, NB_, sim_safe=False):
    import concourse.bacc as bacc
    import concourse.tile as tile
    from concourse import mybir

    F32 = mybir.dt.float32
    F16 = mybir.dt.float16
    AF = mybir.ActivationFunctionType
    ALU = mybir.AluOpType
    # bass_interp has no Erf; Tanh is cost-identical (same engine+table set)
    AF_ERF = AF.Tanh if sim_safe else AF.Erf

    C = T_ // S_
    G4 = 4 * NB_            # one gate-group row block (4 j-blocks x NB)
    SG = S_ * NB_           # gemm moving width per j

    nc = bacc.Bacc("TRN2", target_bir_lowering=False, debug=False,
                   enable_asserts=False)

    xT = nc.dram_tensor("xT", [512, T_ * NB_], F16, kind="ExternalInput").ap()
    wih1 = nc.dram_tensor("wih1", [512, 12 * 128], F16, kind="ExternalInput").ap()
    whh1 = nc.dram_tensor("whh1", [512, 12 * 128], F16, kind="ExternalInput").ap()
    bias1 = nc.dram_tensor("bias1", [128, 12], F32, kind="ExternalInput").ap()
    bnr1 = nc.dram_tensor("bnr1", [1, 512], F16, kind="ExternalInput").ap()
    wih2 = nc.dram_tensor("wih2", [512, 12 * 128], F16, kind="ExternalInput").ap()
    whh2 = nc.dram_tensor("whh2", [512, 12 * 128], F16, kind="ExternalInput").ap()
    bias2 = nc.dram_tensor("bias2", [128, 12], F32, kind="ExternalInput").ap()
    bnr2 = nc.dram_tensor("bnr2", [1, 512], F16, kind="ExternalInput").ap()
    ident = nc.dram_tensor("ident", [128, 128], F16, kind="ExternalInput").ap()
    y = nc.dram_tensor("y", [128, T_ * G4], F16, kind="ExternalOutput").ap()
    y4 = y.rearrange("p (t j b) -> p t j b", j=4, b=NB_)

    with tile.TileContext(nc) as tc, ExitStack() as ctx:
        pools = {}
        for name, bufs in [("wpool", 1), ("xin", 2), ("xp1", 2), ("xp2", 2),
                           ("co1", 2), ("co2", 2), ("mid", 2), ("gate", 3),
                           ("gelu", 2)]:
            pools[name] = ctx.enter_context(tc.tile_pool(name=name, bufs=bufs))
        for name, bufs in [("ps1", 2), ("ps2", 2), ("gps", 2)]:
            pools[name] = ctx.enter_context(
                tc.tile_pool(name=name, bufs=bufs, space="PSUM"))

        def load_w(dram, name):
            t = pools["wpool"].tile([128, 4 * 12 * 128], F16, tag=name)
            for j in range(4):
                nc.sync.dma_start(t[:, j * 12 * 128:(j + 1) * 12 * 128],
                                  dram[j * 128:(j + 1) * 128, :])
            return t

        wih1_sb = load_w(wih1, "wih1")
        whh1_sb = load_w(whh1, "whh1")
        wih2_sb = load_w(wih2, "wih2")
        whh2_sb = load_w(whh2, "whh2")

        def load_t(dram, name, p, w, dt):
            t = pools["wpool"].tile([p, w], dt, tag=name)
            nc.sync.dma_start(t[:], dram[:])
            return t

        bias1_sb = load_t(bias1, "bias1", 128, 12, F32)
        bias2_sb = load_t(bias2, "bias2", 128, 12, F32)
        bnr1_sb = load_t(bnr1, "bnr1", 1, 512, F16)
        bnr2_sb = load_t(bnr2, "bnr2", 1, 512, F16)
        ident_sb = load_t(ident, "ident", 128, 128, F16)
        ones_sb = pools["wpool"].tile([1, NB_], F16, tag="ones")
        nc.vector.memset(ones_sb[:], 1.0)

        # per-layer state: co ring buffers (written once per build below)
        L = {
            1: dict(wih=wih1_sb, whh=whh1_sb, bias=bias1_sb, bnr=bnr1_sb,
                    xp_pool="xp1", co_pool="co1", ps_pool="ps1"),
            2: dict(wih=wih2_sb, whh=whh2_sb, bias=bias2_sb, bnr=bnr2_sb,
                    xp_pool="xp2", co_pool="co2", ps_pool="ps2"),
        }
        for li in (1, 2):
            # co ring: 2 buffers, each [128, (S+1), 4, NB] fp16
            cos = []
            for rb in range(2):
                t = pools[L[li]["co_pool"]].tile([128, (S_ + 1) * G4], F16,
                                                 tag=f"co{li}")
                cos.append(t.rearrange("p (t j b) -> p t j b", j=4, b=NB_))
            L[li]["co"] = cos
            # zero h0 slot (slot S of buffer used before chunk 0, i.e. buf 1)
            nc.vector.memset(cos[1][:, S_, :, :], 0.0)
            L[li]["xp"] = {}

        xin_tiles = {}

        def xin_dma(k):
            if k >= C or k in xin_tiles:
                return
            xs = pools["xin"].tile([128, 4 * SG], F16, tag="xs")
            for j in range(4):
                nc.sync.dma_start(
                    xs[:, j * SG:(j + 1) * SG],
                    xT[j * 128:(j + 1) * 128, k * SG:(k + 1) * SG])
            xin_tiles[k] = xs

        mid_tiles = {}

        def gemm_group(li, k, m, rhs_fn):
            """xp[li][k][m-block] = (sum_j W_ih[j,m].T @ rhs(j)) + bias[m]."""
            d = L[li]
            if k not in d["xp"]:
                t = pools[d["xp_pool"]].tile([128, 12 * SG], F16,
                                             tag=f"xp{li}")
                d["xp"][k] = t.rearrange("p (m t b) -> p m t b", m=12, b=NB_)
            ps = pools["gps"].tile([128, SG], F32, tag="gps")
            for j in range(4):
                nc.tensor.matmul(
                    ps[:], d["wih"][:, (j * 12 + m) * 128:(j * 12 + m + 1) * 128],
                    rhs_fn(j), start=(j == 0), stop=(j == 3))
            # per-partition bias add on ACT; writes fp16 xp
            ps_v = ps.rearrange("p (t b) -> p t b", b=NB_)
            nc.scalar.activation(d["xp"][k][:, m, :, :], ps_v[:],
                                 AF.Identity, bias=d["bias"][:, m:m + 1])

        def rhs1(k):
            xs = xin_tiles[k]
            return lambda j: xs[:, j * SG:(j + 1) * SG]

        def rhs2(k):
            mid4 = mid_tiles[k]
            return lambda j: mid4[:, :, j, :]

        def emit_step(li, k, i):
            d = L[li]
            co_cur = d["co"][k % 2]
            co_prev = d["co"][(k - 1) % 2]
            h_ap = co_cur if i > 0 else co_prev
            h_slot = i if i > 0 else S_
            xp4 = d["xp"][k]
            ps = pools[d["ps_pool"]].tile([128, 12 * NB_], F32, tag=f"ps{li}")
            first = (k == 0 and i == 0)
            m1p, m2p = d.get("m1_prev"), d.get("m2_prev")

            # One accumulation group per burst (start clears the whole 2KB
            # zero region; per-element has_written handles the rest).
            # W @ h_prev is computed as W@m2 + W@m1 with m2 = z*h ready
            # before tanh and m1 = (1-z)*n right after it — so only the
            # m1 matmuls trail the gate chain.
            for m in range(8):
                nc.tensor.matmul(ps[:, m * NB_:(m + 1) * NB_], ident_sb[:],
                                 xp4[:, m, i, :], start=(m == 0), stop=False)
            for m in range(8, 12):
                nc.tensor.matmul(ps[:, m * NB_:(m + 1) * NB_],
                                 d["bnr"][:, (m - 8) * 128:(m - 7) * 128],
                                 ones_sb[:], start=False,
                                 stop=(first and m == 11))
            if not first:
                for m in range(12):
                    sl = ps[:, m * NB_:(m + 1) * NB_]
                    for j in range(4):
                        nc.tensor.matmul(
                            sl,
                            d["whh"][:, (j * 12 + m) * 128:(j * 12 + m + 1) * 128],
                            m2p[:, j, :], start=False, stop=False)
                for m in range(12):
                    sl = ps[:, m * NB_:(m + 1) * NB_]
                    for j in range(4):
                        nc.tensor.matmul(
                            sl,
                            d["whh"][:, (j * 12 + m) * 128:(j * 12 + m + 1) * 128],
                            m1p[:, j, :], start=False,
                            stop=(m == 11 and j == 3))

            g = pools["gate"]
            rz = g.tile([128, 8 * NB_], F32, tag=f"rz{li}")
            nc.scalar.activation(rz[:], ps[:, 0:8 * NB_], AF.Sigmoid)
            t1 = g.tile([128, 4 * NB_], F32, tag=f"t1{li}")
            nc.vector.tensor_mul(t1[:], rz[:, 0:4 * NB_],
                                 ps[:, 8 * NB_:12 * NB_])
            t2 = g.tile([128, 4 * NB_], F32, tag=f"t2{li}")
            nc.vector.tensor_add(t2[:], t1[:], xp4[:, 8:12, i, :])
            nn_ = g.tile([128, 4 * NB_], F32, tag=f"nn{li}")
            nc.scalar.activation(nn_[:], t2[:], AF.Tanh)
            m2t = g.tile([128, 4 * NB_], F16, tag=f"m2{li}")
            nc.vector.tensor_mul(m2t[:], rz[:, 4 * NB_:8 * NB_],
                                 h_ap[:, h_slot, :, :])
            v = g.tile([128, 4 * NB_], F32, tag=f"v{li}")
            nc.vector.tensor_mul(v[:], rz[:, 4 * NB_:8 * NB_], nn_[:])
            m1t = g.tile([128, 4 * NB_], F16, tag=f"m1{li}")
            nc.vector.tensor_sub(m1t[:], nn_[:], v[:])
            nc.vector.tensor_add(co_cur[:, i + 1, :, :], m1t[:], m2t[:])
            d["m1_prev"] = m1t.rearrange("p (j b) -> p j b", b=NB_)
            d["m2_prev"] = m2t.rearrange("p (j b) -> p j b", b=NB_)

        def emit_gelu(k):
            """mid[k] = gelu(co1[k] slots 1..S); erf on ACT, mults on Pool."""
            src = L[1]["co"][k % 2][:, 1:S_ + 1, :, :]
            mid = pools["mid"].tile([128, S_ * G4], F16, tag="mid")
            mid_tiles[k] = mid.rearrange("p (t j b) -> p t j b", j=4, b=NB_)
            erf_t = pools["gelu"].tile([128, S_ * G4], F16, tag="erf")
            nc.scalar.activation(erf_t[:], src, AF_ERF,
                                 scale=0.7071067811865476)
            xe = pools["gelu"].tile([128, S_ * G4], F16, tag="xe")
            nc.vector.scalar_tensor_tensor(xe[:], src, 0.5, erf_t[:],
                                           op0=ALU.mult, op1=ALU.mult)
            nc.vector.scalar_tensor_tensor(
                mid.rearrange("p (t j b) -> p t j b", j=4, b=NB_)[:, :, :, :],
                src, 0.5, xe[:], op0=ALU.mult, op1=ALU.add)

        def emit_ydma(k):
            co = L[2]["co"][k % 2]
            nc.sync.dma_start(y4[:, k * S_:(k + 1) * S_, :, :],
                              co[:, 1:S_ + 1, :, :])

        # ---- main schedule ----
        # L2 lags L1 by 2 chunks: at iteration k, L1 runs chunk k, L2 runs
        # chunk k-2; GELU(k-1) + both layers' GEMM groups for upcoming chunks
        # are spread through the step loop so no chain stalls at chunk tops.
        xin_dma(0)
        xin_dma(1)
        for m in range(12):
            gemm_group(1, 0, m, rhs1(0))

        for k in range(C + 2):
            xin_dma(k + 2)
            if 1 <= k <= C:
                emit_gelu(k - 1)
            # spread slots: L1-GEMM chunk k+1 at i%9==3, L2-GEMM chunk k-1
            # at i%9==7 (needs gelu(k-1) emitted above)
            slots1 = {3 + 9 * m: m for m in range(12)} if k + 1 < C else {}
            slots2 = {7 + 9 * m: m for m in range(12)} if 1 <= k <= C else {}
            for i in range(S_):
                if k < C:
                    emit_step(1, k, i)
                if 2 <= k:
                    emit_step(2, k - 2, i)
                if i in slots1:
                    gemm_group(1, k + 1, slots1[i], rhs1(k + 1))
                if i in slots2:
                    gemm_group(2, k - 1, slots2[i], rhs2(k - 1))
            if k >= 2:
                emit_ydma(k - 2)

    nc.compile()
    return nc


def _get_nc(sim_safe=False):
    key = (T, S, NB, sim_safe)
    if key not in _CACHE:
        _CACHE[key] = _build(T, S, NB, sim_safe)
    return _CACHE[key]


def _prep_core_inputs(x_slice, w_ih1, w_hh1, b_ih1, b_hh1,
                      w_ih2, w_hh2, b_ih2, b_hh2):
    def wstat(w):
        return np.ascontiguousarray(w.T).astype(np.float16)

    def biasv(b_ih, b_hh):
        b = b_ih.astype(np.float64).copy()
        b[:2 * H] += b_hh[:2 * H].astype(np.float64)
        return np.ascontiguousarray(b.reshape(12, 128).T).astype(np.float32)

    def biasnr(b_hh):
        return np.ascontiguousarray(
            b_hh[2 * H:].reshape(1, 512)).astype(np.float16)

    xT = np.ascontiguousarray(
        x_slice.transpose(2, 1, 0).reshape(512, T * NB)).astype(np.float16)
    return {
        "xT": xT,
        "wih1": wstat(w_ih1), "whh1": wstat(w_hh1),
        "bias1": biasv(b_ih1, b_hh1), "bnr1": biasnr(b_hh1),
        "wih2": wstat(w_ih2), "whh2": wstat(w_hh2),
        "bias2": biasv(b_ih2, b_hh2), "bnr2": biasnr(b_hh2),
        "ident": np.eye(128, dtype=np.float16),
    }


def kernel(x, w_ih1, w_hh1, b_ih1, b_hh1, w_ih2, w_hh2, b_ih2, b_hh2):
    from concourse import bass_utils

    x = np.asarray(x, dtype=np.float32)
    args = [np.asarray(a, dtype=np.float32) for a in
            (w_ih1, w_hh1, b_ih1, b_hh1, w_ih2, w_hh2, b_ih2, b_hh2)]

    nc = _get_nc()
    in_maps = [
        _prep_core_inputs(x[c * NB:(c + 1) * NB], *args)
        for c in range(N_CORES)
    ]
    res = bass_utils.run_bass_kernel_spmd(nc, in_maps,
                                          core_ids=list(range(N_CORES)),
                                          trace=TRACE, trace_cores=TRACE_CORES)
    global LAST_RESULTS
    LAST_RESULTS = res
    parts = []
    for c in range(N_CORES):
        yf = res.results[c]["y"].astype(np.float32).reshape(128, T, 4, NB)
        parts.append(np.ascontiguousarray(
            yf.transpose(3, 1, 2, 0).reshape(NB, T, 512)))
    return np.concatenate(parts, axis=0)
